# revision 19
# baseline (speedup 1.0000x reference)
"""CRF negative-log-likelihood loss on 8 Trainium2 NeuronCores.

Math: the forward (log-partition) recurrence
    alpha_{t}[b,j] = logsumexp_i(alpha_{t-1}[b,i] + trans[i,j]) + em[b,t,j]
runs in *scaled linear space*:
    S_t = (E^T @ S_{t-1}) * w_t,   E = exp(trans - CE),  w_t = exp(em_t - CW)
with state S kept transposed ([256 states, as 2x128 partition chunks] x
[batch rows on free]) so each step is four stationary-weight bf16 matmuls
into one PSUM bank plus one vector multiply -- no per-step transpose.
Every RESCALE steps a ones-vector matmul computes per-row column sums s,
the state is scaled by 1/s (broadcast via a K=1 matmul), and the raw s
values are shipped to the host, which re-applies sum(log s) exactly.

Time split via forward/backward meeting:
    Z_b = sum_{k,j} alpha_255[k,b] * E[k,j] * (w*beta)_256[j,b]
The backward half-recurrence H_t = (E^T_rev H_{t+1}) * w_t has the SAME
program as the forward one with transposed transition weights and
time-reversed emissions, so one SPMD program serves both directions.

Sharding: 8 cores = 4 batch groups (8 rows each) x {forward, backward};
each core runs 255 recurrence steps over half the sequence.  The joint
(numerator) score is a 16K-element gather -- ~0.003% of the FLOPs -- done
on the host during unshard, as are the final 256-long dot products and
the mean.  mask is all-ones per the problem spec and only enters the
(host) numerator term.
"""

import numpy as np

B, T, K = 32, 512, 256
NCORES = 8
NGROUP = 4                 # batch groups
RPC = B // NGROUP          # batch rows per core (8)
THALF = T // 2             # 256 time slices per direction
NSTEP = THALF - 1          # 255 recurrence steps per core
# Folds: E = exp(trans - CE), w = exp(em - CW).  Their sum (~log of the
# mean per-step growth of the linear-space state) keeps S near e^0.
CE = 6.0452                # ~ log(K * mean(exp N(0,1))) = log(256) + 0.5
CW = 0.5
RESCALE = 128              # steps between renormalizations
SPLIT_VE = False           # per-J PSUM/S tiles + A-major matmul order
PROBE = None               # None|"nove"|"nomm" - timing probes, break math
TCH = 64                   # emission t-chunk (DMA/exp pipelining)
NCH = THALF // TCH         # 4
NRESC = len(range(RESCALE, NSTEP, RESCALE))   # t = 128 -> 1

TRACE = False
LAST_EXEC_NS = None
LAST_RESULTS = None

_cache = {}


def _build_program(loop_n=None):
    """loop_n=None: the real kernel.  loop_n=R: benchmark build that wraps
    the (loop-invariant) recurrence in a hardware For_i executed R times,
    so device time can be measured as a slope between two R values."""
    key = ("nc", loop_n, SPLIT_VE)
    if key in _cache:
        return _cache[key]
    import concourse.bass as bass
    import concourse.bacc as bacc
    import concourse.mybir as mybir
    import concourse.tile as tile
    from contextlib import ExitStack

    f32 = mybir.dt.float32
    bf16 = mybir.dt.bfloat16
    EXP = mybir.ActivationFunctionType.Exp

    nc = bacc.Bacc("TRN2", target_bir_lowering=False, debug=False)
    # em[p, J, tau, b] = (time-ordered or -reversed) emissions, state J*128+p
    em_dram = nc.dram_tensor("em", [128, 2, THALF, RPC], f32,
                             kind="ExternalInput").ap()
    # trans for forward cores, trans.T for backward cores
    tr_dram = nc.dram_tensor("trans", [K, K], f32, kind="ExternalInput").ap()
    sfin_dram = nc.dram_tensor("sfin", [128, 2, RPC], f32,
                               kind="ExternalOutput").ap()
    sums_dram = nc.dram_tensor("sums", [1, max(NRESC, 1) * RPC], f32,
                               kind="ExternalOutput").ap()

    with tile.TileContext(nc) as tc:
        with ExitStack() as ctx:
            const = ctx.enter_context(tc.tile_pool(name="const", bufs=1))
            stage = ctx.enter_context(tc.tile_pool(name="stage", bufs=2))
            wpool = ctx.enter_context(tc.tile_pool(name="w", bufs=1))
            spool = ctx.enter_context(tc.tile_pool(name="s", bufs=4))
            ps = ctx.enter_context(
                tc.tile_pool(name="ps", bufs=4, space=bass.MemorySpace.PSUM))
            ps_r = ctx.enter_context(
                tc.tile_pool(name="psr", bufs=1, space=bass.MemorySpace.PSUM))
            ps_b = ctx.enter_context(
                tc.tile_pool(name="psb", bufs=1, space=bass.MemorySpace.PSUM))
            ps_w = ctx.enter_context(
                tc.tile_pool(name="psw", bufs=1, space=bass.MemorySpace.PSUM))

            # ---- HAM warmup: keep PE busy while DMA/exp fills SBUF ----
            scratch = const.tile([128, 128], bf16, tag="scratch")
            nc.gpsimd.memset(scratch[:], 0.0)
            warm = ps_w.tile([128, 128], f32, tag="warm")
            for _ in range(48):
                nc.tensor.matmul(warm[:], scratch[:], scratch[:],
                                 start=True, stop=True)

            ones_col = const.tile([128, 1], bf16, tag="ones_col")
            nc.gpsimd.memset(ones_col[:], 1.0)
            ones_row = const.tile([1, 128], f32, tag="ones_row")
            nc.gpsimd.memset(ones_row[:], 1.0)
            bias_e = const.tile([128, 1], f32, tag="bias_e")
            nc.gpsimd.memset(bias_e[:], -CE)
            bias_w = const.tile([128, 1], f32, tag="bias_w")
            nc.gpsimd.memset(bias_w[:], -CW)
            sums_sb = const.tile([1, max(NRESC, 1) * RPC], f32, tag="sums_sb")
            nc.gpsimd.memset(sums_sb[:], 1.0)
            sfin_sb = const.tile([128, 2, RPC], f32, tag="sfin_sb")

            # ---- transition matrix -> E = exp(trans - CE), bf16 ----
            E = []
            for A in range(2):
                tstage = stage.tile([128, K], f32, tag="tstage")
                nc.sync.dma_start(tstage[:], tr_dram[A * 128:(A + 1) * 128, :])
                e = const.tile([128, K], bf16, tag=f"E{A}")
                nc.scalar.activation(e[:], tstage[:], EXP, bias=bias_e[:])
                E.append(e)

            # ---- emissions -> w = exp(em - CW), bf16, chunked on tau ----
            w = [None] * NCH
            for c in range(NCH):
                est = stage.tile([128, 2, TCH, RPC], f32, tag="emstage")
                nc.sync.dma_start(
                    est[:], em_dram[:, :, c * TCH:(c + 1) * TCH, :])
                wt = wpool.tile([128, 2, TCH, RPC], bf16, tag=f"w{c}")
                nc.scalar.activation(wt[:], est[:], EXP, bias=bias_w[:])
                w[c] = wt

            def step_fused(S, c, r):
                """One PSUM bank, one VE op per step."""
                p = ps.tile([128, 2, RPC], f32, tag="ps")
                if PROBE != "nomm":
                    for J in range(2):
                        nc.tensor.matmul(p[:, J, :],
                                         E[0][:, J * 128:(J + 1) * 128],
                                         S[0], start=True, stop=False)
                        nc.tensor.matmul(p[:, J, :],
                                         E[1][:, J * 128:(J + 1) * 128],
                                         S[1], start=False, stop=True)
                if PROBE == "nove":
                    # matmuls feed a throwaway VE op that is NOT on the
                    # next step's input: chain dependency through PE only
                    junk = spool.tile([128, 2, RPC], bf16, tag="junk")
                    nc.vector.tensor_mul(junk[:], p[:], w[c][:, :, r, :])
                    return S, None
                if PROBE == "nomm":
                    Sn = spool.tile([128, 2, RPC], bf16, tag="S")
                    nc.vector.tensor_mul(Sn[:], w[c][:, :, r, :],
                                         w[c][:, :, r, :])
                    return [Sn[:, 0, :], Sn[:, 1, :]], Sn[:]
                Sn = spool.tile([128, 2, RPC], bf16, tag="S")
                nc.vector.tensor_mul(Sn[:], p[:], w[c][:, :, r, :])
                return [Sn[:, 0, :], Sn[:, 1, :]], Sn[:]

            def step_split(S, c, r):
                """Per-J PSUM tiles and VE ops; A-major matmul order so the
                next step's A=0 matmuls wait only on this step's VE(J0)."""
                p = [ps.tile([128, RPC], f32, tag="ps", name=f"p{J}")
                     for J in range(2)]
                nc.tensor.matmul(p[0][:], E[0][:, 0:128], S[0],
                                 start=True, stop=False)
                nc.tensor.matmul(p[1][:], E[0][:, 128:256], S[0],
                                 start=True, stop=False)
                nc.tensor.matmul(p[0][:], E[1][:, 0:128], S[1],
                                 start=False, stop=True)
                nc.tensor.matmul(p[1][:], E[1][:, 128:256], S[1],
                                 start=False, stop=True)
                Sn = []
                for J in range(2):
                    s = spool.tile([128, RPC], bf16, tag=f"S{J}")
                    nc.vector.tensor_mul(s[:], p[J][:], w[c][:, J, r, :])
                    Sn.append(s)
                return [Sn[0][:], Sn[1][:]], None

            step = step_split if SPLIT_VE else step_fused

            def chain_body():
                # ---- initial state S_0 = w[:, :, tau=0, :] ----
                if SPLIT_VE:
                    S = []
                    for A in range(2):
                        s0 = spool.tile([128, RPC], bf16, tag=f"S{A}")
                        nc.vector.tensor_copy(s0[:], w[0][:, A, 0, :])
                        S.append(s0[:])
                else:
                    S0 = spool.tile([128, 2, RPC], bf16, tag="S")
                    nc.vector.tensor_copy(S0[:], w[0][:, :, 0, :])
                    S = [S0[:, 0, :], S0[:, 1, :]]

                n_out = 0
                full = None if SPLIT_VE else S0[:]
                # ---- the 255-step recurrence ----
                for t in range(1, NSTEP + 1):
                    c, r = divmod(t, TCH)
                    S, f2 = step(S, c, r)
                    full = f2 if f2 is not None else full

                    if t % RESCALE == 0 and t < NSTEP:
                        sp = ps_r.tile([1, RPC], f32, tag="colsum")
                        nc.tensor.matmul(sp[:], ones_col[:], S[0],
                                         start=True, stop=False)
                        nc.tensor.matmul(sp[:], ones_col[:], S[1],
                                         start=False, stop=True)
                        # raw column sums -> host (applies log there, f64)
                        nc.vector.tensor_copy(
                            sums_sb[:, n_out * RPC:(n_out + 1) * RPC], sp[:])
                        n_out += 1
                        rec = spool.tile([1, RPC], f32, tag="recip")
                        nc.vector.reciprocal(rec[:], sp[:])
                        bc = ps_b.tile([128, RPC], f32, tag="bc")
                        nc.tensor.matmul(bc[:], ones_row[:], rec[:],
                                         start=True, stop=True)
                        if SPLIT_VE:
                            Sn = []
                            for A in range(2):
                                s = spool.tile([128, RPC], bf16, tag=f"S{A}")
                                nc.vector.tensor_mul(s[:], S[A], bc[:])
                                Sn.append(s[:])
                            S = Sn
                        else:
                            Sr = spool.tile([128, 2, RPC], bf16, tag="S")
                            for A in range(2):
                                nc.vector.tensor_mul(Sr[:, A, :], S[A], bc[:])
                            S = [Sr[:, 0, :], Sr[:, 1, :]]
                            full = Sr[:]

                # ---- ship final state (f32) ----
                if SPLIT_VE:
                    for A in range(2):
                        nc.vector.tensor_copy(sfin_sb[:, A, :], S[A])
                else:
                    nc.vector.tensor_copy(sfin_sb[:], full)

            if loop_n is None:
                chain_body()
            else:
                with tc.For_i(0, loop_n, 1):
                    chain_body()
            nc.sync.dma_start(sfin_dram[:], sfin_sb[:])
            nc.sync.dma_start(sums_dram[:], sums_sb[:])

    nc.compile()
    _cache[key] = nc
    return nc


def _log_numerator(emissions, tags, mask, trans):
    e64 = np.asarray(emissions, np.float64)
    t64 = np.asarray(trans, np.float64)
    tg = np.asarray(tags)
    mk = np.asarray(mask, np.float64)
    emit = np.take_along_axis(e64, tg[:, :, None].astype(np.int64),
                              axis=2)[..., 0]
    score = (emit * mk).sum(1)
    score += (t64[tg[:, :-1], tg[:, 1:]] * mk[:, 1:]).sum(1)
    return score


def _make_in_maps(em, tr):
    """Core 2g   = forward  half (t 0..255, trans),
       core 2g+1 = backward half (t 511..256 reversed, trans.T)."""
    trT = np.ascontiguousarray(tr.T)
    in_maps = []
    for g in range(NGROUP):
        rows = em[g * RPC:(g + 1) * RPC]             # [RPC, T, K]
        fwd = rows[:, :THALF]                        # [RPC, 256, K]
        bwd = rows[:, THALF:][:, ::-1]               # reversed time
        for half, tmat in ((fwd, tr), (bwd, trT)):
            # [RPC, THALF, K] -> [p, J, tau, b] with state = J*128+p
            x = half.transpose(2, 1, 0).reshape(2, 128, THALF, RPC)
            x = np.ascontiguousarray(x.transpose(1, 0, 2, 3))
            in_maps.append({"em": x, "trans": tmat})
    return in_maps


def kernel(emissions, tags, mask, transition_scores):
    global LAST_EXEC_NS, LAST_RESULTS
    from concourse.bass_utils import run_bass_kernel_spmd

    em = np.ascontiguousarray(np.asarray(emissions, np.float32))
    tr = np.ascontiguousarray(np.asarray(transition_scores, np.float32))

    nc = _build_program()
    in_maps = _make_in_maps(em, tr)
    res = run_bass_kernel_spmd(nc, in_maps, core_ids=list(range(NCORES)),
                               trace=TRACE)
    LAST_EXEC_NS = res.exec_time_ns
    LAST_RESULTS = res

    # ---- unshard: combine forward/backward halves per batch group ----
    E64 = np.exp(np.asarray(tr, np.float64) - CE)
    log_den = np.empty(B, np.float64)
    for g in range(NGROUP):
        parts = []
        for d in range(2):
            r = res.results[2 * g + d]
            s = np.asarray(r["sfin"], np.float64).reshape(128, 2, RPC)
            s = s.transpose(1, 0, 2).reshape(K, RPC)          # [state, b]
            gam = np.log(np.asarray(r["sums"], np.float64)
                         .reshape(NRESC, RPC)).sum(0)
            parts.append((s, gam))
        (sf, gf), (sb, gb) = parts
        z = np.einsum("kb,kj,jb->b", sf, E64, sb)
        log_den[g * RPC:(g + 1) * RPC] = (
            np.log(z) + gf + gb
            + 2 * CW + 2 * NSTEP * (CE + CW) + CE)

    log_num = _log_numerator(emissions, tags, mask, transition_scores)
    return np.float32(np.mean(log_den - log_num))
